# revision 2
# baseline (speedup 1.0000x reference)
"""Trainium2 Bass kernel for a 2-layer GCN (GCNConv -> ReLU -> GCNConv).

Math (reference):
    add self-loops; deg = indegree (unit weights); dis = deg^-1/2
    norm_e = dis[row_e] * dis[col_e]
    h   = relu( segsum_col( (x @ W1)[row] * norm ) + b1 )
    out =       segsum_col( (h @ W2)[row] * norm ) + b2

Kernel reorganization (linearity of segment-sum):
    agg1[d] = sum_e norm_e * x[row_e]        (segment-sum of raw feature rows)
    h[d]    = relu( agg1[d] @ W1 + b1 )
    hw[v]   = h[v] @ W2                      (computed right after h, per tile)
    out[d]  = sum_e norm_e * hw[row_e] + b2

Distribution (8 cores, SPMD shared program): destinations sharded across
cores; both layers are dest-sharded gathers + on-chip segment reduction.
Layer 1 gathers 512B x-rows from the replicated input table.  Layer 2
gathers 256B rows of a bf16 hw table that is AllGather'd (8MB wire as
unpadded [T*P, 40] bf16) and locally repacked to 256B row stride.

Gather engine: gpsimd.dma_gather (custom SWDGE ucode).  int16 indices =>
tables are split into 32768-row banks; each (position, bank) group is
regularized to a shared chunk count (max over cores at the same rank) so
the single SPMD program fits all cores.  Calls are <=1024 indices (SWDGE
ring capacity) and round-robin over 4 SWDGE queues (parallel desc-gen).

Per-core segment reduction for one tile of 128 destinations:
    For each chunk of 128 edges (grouped by dest tile on host):
      SelT[e, d] = (iota[d] == colrel[e]) * norm[e]     -- one DVE tensor_scalar
      PSUM[feat, dest] += gathered[e, feat]^T @ SelT    -- PE matmul, K=e
    then out1t[H,dest] = W1^T @ agg (+b1), relu, hw = h^T @ W2, DMA out.
"""

import math
import os
import sys

for _p in ("/opt/trn_rl_repo", "/root/.axon_site/_ro/trn_rl_repo"):
    if os.path.isdir(_p) and _p not in sys.path:
        sys.path.insert(0, _p)

import numpy as np

P = 128
BK = 32768           # int16 bank rows
CALL_SLOTS = 8       # max slots (of 128 edges) per dma_gather call
NQ = 4               # SWDGE queues


class Plan:
    pass


class LayerPlan:
    pass


def _layer_layout(owner, pos, bank, counts_cib, T, NB, M, batch_cap):
    """Build the slot stream for one layer.

    counts_cib: [M, T, NB] per-core edge counts.
    Returns (S, slot_lo[T, NB], batches) where batches is a list of dicts:
      pos_lo, pos_hi, slot_lo, slot_hi,
      calls: [(slot_lo, nslots, bank)],
      pos_chunks: {i: [(slot_lo, nslots)]} accumulation ranges per position.
    """
    cib = np.maximum(0, -(-counts_cib.max(axis=0) // P))  # [T, NB]
    # every position needs >= 1 slot total (guaranteed by self-loops, but be safe)
    for i in range(T):
        if cib[i].sum() == 0:
            cib[i][0] = 1
    pos_tot = cib.sum(axis=1)  # slots per position

    batches = []
    slot_lo_arr = np.zeros((T, NB), dtype=np.int64)
    gslot = 0
    i = 0
    while i < T:
        j = i + 1
        tot = pos_tot[i]
        while j < T and tot + pos_tot[j] <= batch_cap:
            tot += pos_tot[j]
            j += 1
        b0 = {"pos_lo": i, "pos_hi": j, "slot_lo": gslot,
              "calls": [], "pos_chunks": {k: [] for k in range(i, j)}}
        for b in range(NB):
            run_lo = gslot
            for k in range(i, j):
                n = int(cib[k, b])
                if n == 0:
                    continue
                slot_lo_arr[k, b] = gslot
                b0["pos_chunks"][k].append((gslot, n))
                gslot += n
            # split the bank run into <=CALL_SLOTS calls
            r = run_lo
            while r < gslot:
                n = min(CALL_SLOTS, gslot - r)
                b0["calls"].append((r, n, b))
                r += n
        b0["slot_hi"] = gslot
        batches.append(b0)
        i = j
    return int(gslot), slot_lo_arr, batches, cib


def _fill_layer_arrays(lp, M, T, NB, owner, pos, bank, lidx, colrel, normv, order_key):
    """Scatter per-edge metadata into the slot/lane arrays."""
    S = lp.S
    E2 = owner.shape[0]
    blockid = (owner * T + pos) * NB + bank
    counts = np.bincount(blockid, minlength=M * T * NB)
    order = np.argsort(blockid, kind="stable")
    sb = blockid[order]
    starts = np.zeros(M * T * NB + 1, dtype=np.int64)
    np.cumsum(counts, out=starts[1:])
    q = np.arange(E2, dtype=np.int64) - starts[sb]
    o_pos = pos[order]
    o_bank = bank[order]
    slot = lp.slot_lo[o_pos, o_bank] + q // P
    lane = q % P

    crnorm = np.zeros((M, P, 2 * S), dtype=np.float32)
    crnorm[:, :, 0:S] = -1.0
    g16 = np.zeros((M, 16, 8 * S), dtype=np.int16)
    o_owner = owner[order]
    e = slot * P + lane
    crnorm[o_owner, lane, slot] = colrel[order]
    crnorm[o_owner, lane, S + slot] = normv[order]
    g16[o_owner, e % 16, e // 16] = lidx[order]
    lp.crnorm = crnorm
    lp.gidx16 = np.tile(g16, (1, 8, 1))  # replicate to 128 partitions


def make_plan(edge_index, n_nodes, n_cores, f_in, hidden, n_class,
              l1_batch_cap=48, l2_batch_cap=96):
    pl = Plan()
    N = n_nodes
    M = n_cores
    row = np.asarray(edge_index[0], dtype=np.int64)
    col = np.asarray(edge_index[1], dtype=np.int64)
    loops = np.arange(N, dtype=np.int64)
    row_all = np.concatenate([row, loops])
    col_all = np.concatenate([col, loops])

    deg = np.bincount(col_all, minlength=N).astype(np.float32)
    dis = (1.0 / np.sqrt(np.maximum(deg, 1e-12))).astype(np.float32)
    dis[deg <= 0] = 0.0
    normv = dis[row_all] * dis[col_all]

    Nc = -(-N // M)
    T = -(-Nc // P)
    owner = col_all // Nc
    local = col_all - owner * Nc
    ltile = local // P
    colrel = (local - ltile * P).astype(np.float32)

    counts = np.bincount(owner * T + ltile, minlength=M * T).reshape(M, T)
    perm = np.argsort(-counts, axis=1, kind="stable")
    posidx = np.empty_like(perm)
    for c in range(M):
        posidx[c, perm[c]] = np.arange(T)
    e_pos = posidx[owner, ltile]

    v = np.arange(N, dtype=np.int64)
    v_owner = v // Nc
    v_local = v - v_owner * Nc
    v_tile = v_local // P
    ghwrow = (v_owner * (T * P) + posidx[v_owner, v_tile] * P
              + (v_local - v_tile * P)).astype(np.int64)
    HWROWS = M * T * P

    def layer(rows_of_edge, nrows, batch_cap):
        lp = LayerPlan()
        NB = -(-nrows // BK)
        bank = rows_of_edge // BK
        lidx = (rows_of_edge - bank * BK).astype(np.int16)
        cc = np.zeros((M, T, NB), dtype=np.int64)
        np.add.at(cc, (owner, e_pos, bank), 1)
        lp.NB = NB
        lp.S, lp.slot_lo, lp.batches, lp.cib = _layer_layout(
            owner, e_pos, bank, cc, T, NB, M, batch_cap)
        _fill_layer_arrays(lp, M, T, NB, owner, e_pos, bank, lidx,
                           colrel, normv, None)
        return lp

    pl.N, pl.M, pl.Nc, pl.T = N, M, Nc, T
    pl.F, pl.H, pl.C = f_in, hidden, n_class
    pl.HWROWS = HWROWS
    pl.ghwrow = ghwrow
    pl.l1 = layer(row_all, N, l1_batch_cap)
    pl.l2 = layer(ghwrow[row_all], HWROWS, l2_batch_cap)
    return pl


# ---------------------------------------------------------------------------
# Device program
# ---------------------------------------------------------------------------
def build_program(pl, debug=False, debug_mode=None):
    from concourse import bass, bacc, mybir
    import concourse.tile as tile
    from contextlib import ExitStack

    f32 = mybir.dt.float32
    bf16 = mybir.dt.bfloat16
    i32 = mybir.dt.int32
    i16 = mybir.dt.int16
    N, M, T = pl.N, pl.M, pl.T
    F, H, C = pl.F, pl.H, pl.C
    HWROWS = pl.HWROWS
    S1, S2 = pl.l1.S, pl.l2.S

    nc = bacc.Bacc("TRN2", target_bir_lowering=False, debug=debug,
                   num_devices=M, num_swdge_queues=NQ)
    x_p = nc.declare_dram_parameter("x", [N, F], f32, isOutput=False)
    w1_p = nc.declare_dram_parameter("W1", [F, H], f32, isOutput=False)
    b1_p = nc.declare_dram_parameter("b1", [1, H], f32, isOutput=False)
    w2_p = nc.declare_dram_parameter("W2", [H, C], f32, isOutput=False)
    b2_p = nc.declare_dram_parameter("b2", [1, C], f32, isOutput=False)
    crn1_p = nc.declare_dram_parameter("crn1", [P, 2 * S1], f32, isOutput=False)
    g16_1_p = nc.declare_dram_parameter("g16_1", [P, 8 * S1], i16, isOutput=False)
    crn2_p = nc.declare_dram_parameter("crn2", [P, 2 * S2], f32, isOutput=False)
    g16_2_p = nc.declare_dram_parameter("g16_2", [P, 8 * S2], i16, isOutput=False)
    out_p = nc.declare_dram_parameter("out", [T * P, C], f32, isOutput=True)

    hw_ag_in = nc.dram_tensor("hw_ag_in", [T * P, C], bf16)
    hw_ag_out = nc.dram_tensor("hw_ag_out", [HWROWS, C], bf16, addr_space="Shared")
    hw_tab = nc.dram_tensor("hw_tab", [HWROWS, P], bf16)

    qrr = [0]

    def next_q():
        q = qrr[0]
        qrr[0] = (q + 1) % NQ
        return q

    with tile.TileContext(nc) as tc, ExitStack() as ctx:
        const = ctx.enter_context(tc.tile_pool(name="const", bufs=1))

        iota_i = const.tile([P, P], i32)
        iota_f = const.tile([P, P], f32)
        nc.gpsimd.iota(iota_i[:], pattern=[[1, P]], base=0, channel_multiplier=0)
        nc.vector.tensor_copy(out=iota_f[:], in_=iota_i[:])
        ones_1 = const.tile([1, P], f32)
        nc.vector.memset(ones_1[:], 1.0)
        zbias = const.tile([P, 1], f32)
        nc.vector.memset(zbias[:], 0.0)

        w1_sb = const.tile([F, H], f32)
        b1_sb = const.tile([1, H], f32)
        w2_sb = const.tile([H, C], f32)
        b2_sb = const.tile([1, C], f32)
        nc.sync.dma_start(out=w1_sb[:], in_=w1_p[:, :])
        nc.sync.dma_start(out=b1_sb[:], in_=b1_p[:, :])
        nc.sync.dma_start(out=w2_sb[:], in_=w2_p[:, :])
        nc.sync.dma_start(out=b2_sb[:], in_=b2_p[:, :])

        def sel_build(pool, crnorm_sb, S, slot, dt):
            selT = pool.tile([P, P], dt, name="selT")
            nc.vector.tensor_scalar(
                out=selT[:],
                in0=iota_f[:],
                scalar1=crnorm_sb[:, slot:slot + 1],
                scalar2=crnorm_sb[:, S + slot:S + slot + 1],
                op0=mybir.AluOpType.is_equal,
                op1=mybir.AluOpType.mult,
            )
            return selT

        def gather_batch(gp, bat, g16_sb, table_ap, elem, dt, ebytes):
            nb = bat["slot_hi"] - bat["slot_lo"]
            gbuf = gp.tile([P, nb * elem], dt, tag="gbuf")
            for (slo, nsl, b) in (bat["calls"] if debug_mode != "nogather" else []):
                ni = nsl * P
                lo = slo - bat["slot_lo"]
                nc.gpsimd.dma_gather(
                    out_ap=gbuf[:, lo * elem:(lo + nsl) * elem]
                        .rearrange("p (c f) -> p c f", f=elem),
                    in_ap=table_ap(b),
                    idxs_ap=g16_sb[:, slo * 8:(slo + nsl) * 8],
                    num_idxs=ni, num_idxs_reg=ni, elem_size=elem,
                    queue_num=next_q(),
                )
            return gbuf

        # ---------------- layer 1 ----------------
        with tc.tile_pool(name="l1meta", bufs=1) as l1m, \
             tc.tile_pool(name="l1gather", bufs=2) as gp, \
             tc.tile_pool(name="l1sel", bufs=4) as selp, \
             tc.tile_pool(name="l1work", bufs=3) as wp, \
             tc.tile_pool(name="l1agg_ps", bufs=2, space="PSUM") as agg_ps, \
             tc.tile_pool(name="l1o1_ps", bufs=2, space="PSUM") as o1_ps, \
             tc.tile_pool(name="l1hw_ps", bufs=2, space="PSUM") as hw_ps:
            crn1_sb = l1m.tile([P, 2 * S1], f32, name="crn1_sb")
            g16_1_sb = l1m.tile([P, 8 * S1], i16, name="g16_1_sb")
            nc.sync.dma_start(out=crn1_sb[:], in_=crn1_p[:, :])
            nc.sync.dma_start(out=g16_1_sb[:], in_=g16_1_p[:, :])

            def x_table(b):
                return x_p[b * BK:min((b + 1) * BK, N), :]

            for bat in pl.l1.batches:
                gbuf = gather_batch(gp, bat, g16_1_sb, x_table, F, f32, 4)
                for i in range(bat["pos_lo"],
                               bat["pos_hi"] if debug_mode != "gatheronly"
                               else bat["pos_lo"]):
                    psum_agg = agg_ps.tile([P, P], f32, name="psum_agg")
                    ranges = bat["pos_chunks"][i]
                    tot = sum(n for (_, n) in ranges)
                    done = 0
                    for (slo, n) in ranges:
                        for j in range(n):
                            slot = slo + j
                            selT = sel_build(selp, crn1_sb, S1, slot, f32)
                            cofs = (slot - bat["slot_lo"]) * F
                            nc.tensor.matmul(
                                out=psum_agg[:],
                                lhsT=gbuf[:, cofs:cofs + F],
                                rhs=selT[:],
                                start=(done == 0),
                                stop=(done == tot - 1),
                            )
                            done += 1
                    agg_sb = wp.tile([P, P], f32, name="agg_sb")
                    nc.vector.tensor_copy(out=agg_sb[:], in_=psum_agg[:])
                    psum_o1 = o1_ps.tile([H, P], f32, name="psum_o1")
                    nc.tensor.matmul(out=psum_o1[:], lhsT=w1_sb[:],
                                     rhs=agg_sb[:], start=True, stop=False)
                    nc.tensor.matmul(out=psum_o1[:], lhsT=b1_sb[:],
                                     rhs=ones_1[:], start=False, stop=True)
                    h_sb = wp.tile([H, P], f32, name="h_sb")
                    nc.scalar.activation(
                        h_sb[:], psum_o1[:],
                        mybir.ActivationFunctionType.Relu, bias=zbias[:])
                    psum_hw = hw_ps.tile([P, C], f32, name="psum_hw")
                    nc.tensor.matmul(out=psum_hw[:], lhsT=h_sb[:],
                                     rhs=w2_sb[:], start=True, stop=True)
                    hw_sb = wp.tile([P, C], bf16, name="hw_sb")
                    nc.vector.tensor_copy(out=hw_sb[:], in_=psum_hw[:])
                    nc.sync.dma_start(
                        out=(out_p if debug_mode == "hw" else hw_ag_in)
                        [i * P:(i + 1) * P, :], in_=hw_sb[:])

        if debug_mode != "hw":
            # ------------- all-gather + repack -------------
            if debug_mode != "nocc":
                nc.gpsimd.collective_compute(
                    "AllGather",
                    mybir.AluOpType.bypass,
                    replica_groups=[list(range(M))],
                    ins=[hw_ag_in[:, :]],
                    outs=[hw_ag_out[:, :]],
                )
            for rb in range(0, HWROWS, BK):
                re_ = min(rb + BK, HWROWS)
                nc.sync.dma_start(out=hw_tab[rb:re_, 0:C],
                                  in_=hw_ag_out[rb:re_, :])

            # ---------------- layer 2 ----------------
            with tc.tile_pool(name="l2meta", bufs=1) as l2m, \
                 tc.tile_pool(name="l2gather", bufs=2) as gp2, \
                 tc.tile_pool(name="l2sel", bufs=4) as selp2, \
                 tc.tile_pool(name="l2work", bufs=3) as wp2, \
                 tc.tile_pool(name="l2o2_ps", bufs=4, space="PSUM") as o2_ps:
                crn2_sb = l2m.tile([P, 2 * S2], f32, name="crn2_sb")
                g16_2_sb = l2m.tile([P, 8 * S2], i16, name="g16_2_sb")
                nc.sync.dma_start(out=crn2_sb[:], in_=crn2_p[:, :])
                nc.sync.dma_start(out=g16_2_sb[:], in_=g16_2_p[:, :])

                def hw_table(b):
                    return hw_tab[b * BK:min((b + 1) * BK, HWROWS), :]

                for bat in pl.l2.batches:
                    gbuf2 = gather_batch(gp2, bat, g16_2_sb, hw_table, P, bf16, 2)
                    for i in range(bat["pos_lo"],
                                   bat["pos_hi"] if debug_mode != "gatheronly"
                                   else bat["pos_lo"]):
                        psum_o2 = o2_ps.tile([P, C], f32, name="psum_o2")
                        for (slo, n) in bat["pos_chunks"][i]:
                            for j in range(n):
                                slot = slo + j
                                selT = sel_build(selp2, crn2_sb, S2, slot, bf16)
                                cofs = (slot - bat["slot_lo"]) * P
                                nc.tensor.matmul(
                                    out=psum_o2[:],
                                    lhsT=selT[:],
                                    rhs=gbuf2[:, cofs:cofs + C],
                                    start=(slot == bat["pos_chunks"][i][0][0]
                                           and j == 0),
                                    stop=False,
                                )
                        nc.tensor.matmul(out=psum_o2[:], lhsT=ones_1[:],
                                         rhs=b2_sb[:], start=False, stop=True)
                        o_sb = wp2.tile([P, C], f32, name="o_sb")
                        nc.vector.tensor_copy(out=o_sb[:], in_=psum_o2[:])
                        nc.sync.dma_start(
                            out=out_p[i * P:(i + 1) * P, :], in_=o_sb[:])
                if debug_mode == "gatheronly":
                    tok = wp2.tile([P, C], f32, name="o_sb")
                    nc.vector.memset(tok[:], 0.0)
                    nc.sync.dma_start(out=out_p[0:P, :], in_=tok[:])

    nc.compile()
    return nc


# ---------------------------------------------------------------------------
# Input packing / output unpacking
# ---------------------------------------------------------------------------
def make_in_maps(pl, x, W1, b1, W2, b2):
    x = np.ascontiguousarray(np.asarray(x, dtype=np.float32))
    W1 = np.ascontiguousarray(np.asarray(W1, dtype=np.float32))
    b1 = np.ascontiguousarray(np.asarray(b1, dtype=np.float32)).reshape(1, -1)
    W2 = np.ascontiguousarray(np.asarray(W2, dtype=np.float32))
    b2 = np.ascontiguousarray(np.asarray(b2, dtype=np.float32)).reshape(1, -1)
    in_maps = []
    for c in range(pl.M):
        in_maps.append({
            "x": x,
            "W1": W1, "b1": b1, "W2": W2, "b2": b2,
            "crn1": np.ascontiguousarray(pl.l1.crnorm[c]),
            "g16_1": np.ascontiguousarray(pl.l1.gidx16[c]),
            "crn2": np.ascontiguousarray(pl.l2.crnorm[c]),
            "g16_2": np.ascontiguousarray(pl.l2.gidx16[c]),
        })
    return in_maps


def unpack_outputs(pl, outs):
    allout = np.concatenate([np.asarray(o) for o in outs], axis=0)
    return np.ascontiguousarray(allout[pl.ghwrow])


# ---------------------------------------------------------------------------
# Public entry point
# ---------------------------------------------------------------------------
_CACHE = {}


def _get_compiled(edge_index, n_nodes, f_in, hidden, n_class, n_cores=8):
    key = (edge_index.shape, n_nodes, f_in, hidden, n_class, n_cores,
           int(np.asarray(edge_index[0, :8]).sum()),
           int(np.asarray(edge_index[1, -8:]).sum()))
    hit = _CACHE.get(key)
    if hit is None:
        pl = make_plan(edge_index, n_nodes, n_cores, f_in, hidden, n_class)
        nc = build_program(pl)
        _CACHE[key] = hit = (pl, nc)
    return hit


def kernel(x, edge_index, W1, b1, W2, b2):
    from concourse import bass_utils

    x = np.asarray(x)
    edge_index = np.asarray(edge_index)
    n_nodes, f_in = x.shape
    hidden = np.asarray(W1).shape[1]
    n_class = np.asarray(W2).shape[1]
    n_cores = 8

    pl, nc = _get_compiled(edge_index, n_nodes, f_in, hidden, n_class, n_cores)
    in_maps = make_in_maps(pl, x, W1, b1, W2, b2)
    res = bass_utils.run_bass_kernel_spmd(
        nc, in_maps, core_ids=list(range(n_cores)))
    kernel.last_exec_time_ns = res.exec_time_ns
    kernel.last_results = res
    outs = [res.results[c]["out"] for c in range(n_cores)]
    out = unpack_outputs(pl, outs)[:n_nodes]
    return out

